# revision 2
# baseline (speedup 1.0000x reference)
"""BFP activation quantization kernel for 8 TRN2 NeuronCores.

Reference semantics (per (n,h,w) block over C=256 channels):
    max_abs = max_c |x|;  e: max_abs = m*2^e, m in [0.5,1)  (frexp)
    delta = 2^(e-3);  q = trunc(x / delta) * delta

Sharding: pure data-parallel over N (batch): 64 images -> 8 per core.

Key insight: trunc(x/delta) keeps at most the top 2 mantissa bits of x
(|x/delta| < 8), so the bf16 truncation xb of x yields an EXACT result.
The whole datapath runs 16-bit after one 32-bit extraction pass:
  E1  bits' = (bits(x) >>a 16) | 1    (DVE, in-place i32). The |1 makes
      u = xb/delta never an exact integer (OR cannot carry, so sign,
      exponent and the block max's exponent are unchanged), which makes
      the single round-to-nearest convert below exact at trunc.
  E2  s16 = cvt_i16(bits')            (scalar engine; |v| < 2^15 exact)
  sb  = [s16 < 0]                     (DVE is_lt; int16 shifts of
      sign-extended values are NOT trustworthy on the 32-bit ALU lanes)
  a   = s16 & 0x7fff  -> fold halves with max -> ONE partition_all_reduce
      (max) per chunk at [128, HF] bf16. Folding first halves the Q7
      reduce width: the reduce cost is superlinear in width on HW and
      was the dominant bottleneck at [128, 2*HF].
  eb  = bits16(mx) & 0x7f80;  nr = -eb (= bits of -recip, exact int16
      negate since bits16(-2^k) = -bits16(2^k) mod 2^16);  db = eb-0x100
      (= bits of delta)
  t   = xb * (-recip)   (bf16 tensor_tensor, 2x DVE mode; the product is
      exact: 8-bit significand times a power of two)
  r   = cvt_i16(t + c), c = 0.5 - 2^-25  (scalar engine; equals ceil(t)
      because u is never an integer)
  w   = sb - r = trunc(u);  q = w * delta -> bf16 (exact: |w| <= 7
      times a power of two), converted to fp32 on the host (exact).
Verified bit-identical to the reference on the full input.
"""

import sys

for _p in ("/opt/trn_rl_repo", "/root/.axon_site/_ro/trn_rl_repo"):
    if _p not in sys.path:
        sys.path.append(_p)

import numpy as np

N, C, H, W = 64, 256, 56, 56
HW = H * W
NCORES = 8
NPC = N // NCORES  # images per core
F = 3136
HF = F // 2  # columns per chunk (2 chunks per image)
OB, OC, OD, OE, OF, OG, OH = 1, 2, 3, 4, 5, 6, 7  # stage offsets (chunks)
B_XT, B_S16, B_SB, B_RR, B_MXW, B_MXF, B_MFF, B_NR, B_DB = (
    4, 6, 5, 4, 3, 3, 3, 3, 5,
)

_cache = {}


def _build(repeat=1):
    key = ("v5", repeat)
    if key in _cache:
        return _cache[key]

    import concourse.bacc as bacc
    import concourse.mybir as mybir
    import concourse.tile as tile
    from concourse import bass_isa

    dt = mybir.dt
    op = mybir.AluOpType

    nc = bacc.Bacc(
        "TRN2",
        target_bir_lowering=False,
        debug=False,
        enable_asserts=False,
        num_devices=NCORES,
    )
    x_d = nc.dram_tensor("x", [NPC, C, HW], dt.float32, kind="ExternalInput").ap()
    y_d = nc.dram_tensor(
        "y", [NPC, C, HW], dt.bfloat16, kind="ExternalOutput"
    ).ap()

    NCH = NPC * 2 * repeat

    with tile.TileContext(nc) as tc:
        with (
            tc.tile_pool(name="xtp", bufs=B_XT) as xtp,
            tc.tile_pool(name="s16p", bufs=B_S16) as s16p,
            tc.tile_pool(name="sbp", bufs=B_SB) as sbp,
            tc.tile_pool(name="rrp", bufs=B_RR) as rrp,
            tc.tile_pool(name="mxwp", bufs=B_MXW) as mxwp,
            tc.tile_pool(name="mxfp", bufs=B_MXF) as mxfp,
            tc.tile_pool(name="mffp", bufs=B_MFF) as mffp,
            tc.tile_pool(name="nrp", bufs=B_NR) as nrp,
            tc.tile_pool(name="dbp", bufs=B_DB) as dbp,
            tc.tile_pool(name="consts", bufs=1) as consts,
        ):
            cbias = consts.tile([128, 1], dt.float32)  # +(0.5 - 2^-25)
            nc.gpsimd.memset(cbias[:], 0.4999999701976776123046875)

            xts, s16s, sbs, rrs, mxws, mxfs, mffs, nrs, dbs = (
                {}, {}, {}, {}, {}, {}, {}, {}, {},
            )

            def addr(k):
                n = (k // 2) % NPC
                c0 = (k % 2) * HF
                return n, c0

            yv = y_d.rearrange("n (j p) w -> n p j w", j=2)

            for k in range(NCH + OH):
                if OB <= k < NCH + OB:  # E1: (bits >>a 16) | 1 in place
                    j = k - OB
                    xi = xts[j][:].bitcast(dt.int32)
                    nc.vector.tensor_scalar(
                        out=xi, in0=xi, scalar1=16, scalar2=1,
                        op0=op.arith_shift_right, op1=op.bitwise_or,
                    )
                if OC <= k < NCH + OC:  # E2: narrow to i16 on scalar engine
                    j = k - OC
                    s16 = s16s[j] = s16p.tile(
                        [128, 2 * HF], dt.int16, tag="s16", name=f"s16_{j}"
                    )
                    nc.scalar.copy(out=s16[:], in_=xts[j][:].bitcast(dt.int32))
                if OD <= k < NCH + OD:  # sb, |bits|, fold, Q7 reduce
                    j = k - OD
                    s16 = s16s[j]
                    sb = sbs[j] = sbp.tile(
                        [128, 2 * HF], dt.int16, tag="sb", name=f"sb{j}"
                    )
                    nc.vector.tensor_scalar(
                        out=sb[:], in0=s16[:], scalar1=0, scalar2=None,
                        op0=op.is_lt,
                    )
                    mxw = mxws[j] = mxwp.tile(
                        [128, 2 * HF], dt.int16, tag="mxw", name=f"mxw{j}"
                    )
                    nc.vector.tensor_scalar(
                        out=mxw[:], in0=s16[:], scalar1=0x7FFF,
                        scalar2=None, op0=op.bitwise_and,
                    )
                    mff = mffs[j] = mffp.tile(
                        [128, HF], dt.int16, tag="mff", name=f"mff{j}"
                    )
                    nc.vector.tensor_tensor(
                        out=mff[:], in0=mxw[:, 0:HF],
                        in1=mxw[:, HF : 2 * HF], op=op.max,
                    )
                    mxf = mxfs[j] = mxfp.tile(
                        [128, HF], dt.bfloat16, tag="mxf", name=f"mxf{j}"
                    )
                    nc.gpsimd.partition_all_reduce(
                        mxf[:], mff[:].bitcast(dt.bfloat16), 128,
                        bass_isa.ReduceOp.max,
                    )
                if OE <= k < NCH + OE:  # eb, nr, db
                    j = k - OE
                    mxf = mxfs[j]
                    mffs.pop(j, None)
                    eb = mxf[:].bitcast(dt.int16)
                    nc.vector.tensor_scalar(
                        out=eb, in0=eb, scalar1=0x7F80, scalar2=None,
                        op0=op.bitwise_and,
                    )
                    nr = nrs[j] = nrp.tile(
                        [128, HF], dt.int16, tag="nr", name=f"nr{j}"
                    )
                    nc.scalar.mul(out=nr[:], in_=eb, mul=-1.0)
                    db = dbs[j] = dbp.tile(
                        [128, HF], dt.int16, tag="db", name=f"db{j}"
                    )
                    nc.vector.tensor_scalar(
                        out=db[:], in0=eb, scalar1=0x100, scalar2=None,
                        op0=op.subtract,
                    )
                if OF <= k < NCH + OF:  # t = xb * (-recip)
                    j = k - OF
                    s16, nr = s16s[j], nrs[j]
                    xb3 = s16[:].rearrange("p (j w) -> p j w", j=2).bitcast(
                        dt.bfloat16
                    )
                    nrf = (
                        nr[:].bitcast(dt.bfloat16)[:, None, :]
                        .broadcast_to([128, 2, HF])
                    )
                    nc.vector.tensor_tensor(
                        out=xb3, in0=xb3, in1=nrf, op=op.mult
                    )
                if OG <= k < NCH + OG:  # r = cvt_i16(t + c)
                    j = k - OG
                    rr = rrs[j] = rrp.tile(
                        [128, 2 * HF], dt.int16, tag="rr", name=f"rr{j}"
                    )
                    nc.scalar.activation(
                        out=rr[:], in_=s16s[j][:].bitcast(dt.bfloat16),
                        func=mybir.ActivationFunctionType.Identity,
                        bias=cbias[:], scale=1.0,
                    )
                if k < NCH:  # loads
                    n, c0 = addr(k)
                    xt = xts[k] = xtp.tile(
                        [128, 2 * HF], dt.float32, tag="xt", name=f"xt{k}"
                    )
                    nc.sync.dma_start(
                        out=xt[:, 0:HF], in_=x_d[n, 0:128, c0 : c0 + HF]
                    )
                    nc.sync.dma_start(
                        out=xt[:, HF : 2 * HF], in_=x_d[n, 128:256, c0 : c0 + HF]
                    )
                if OH <= k:  # w = sb - r ; q = w * delta ; store
                    j = k - OH
                    if j >= NCH:
                        continue
                    n, c0 = addr(j)
                    xts.pop(j, None)
                    s16s.pop(j)
                    sb = sbs.pop(j)
                    rr = rrs.pop(j)
                    mxws.pop(j, None)
                    mxfs.pop(j, None)
                    nrs.pop(j, None)
                    db = dbs.pop(j)
                    nc.vector.tensor_tensor(
                        out=sb[:], in0=sb[:], in1=rr[:], op=op.subtract
                    )
                    dbf = (
                        db[:].bitcast(dt.bfloat16)[:, None, :]
                        .broadcast_to([128, 2, HF])
                    )
                    q3 = rr[:].rearrange("p (j w) -> p j w", j=2).bitcast(
                        dt.bfloat16
                    )
                    nc.vector.tensor_tensor(
                        out=q3,
                        in0=sb[:].rearrange("p (j w) -> p j w", j=2),
                        in1=dbf, op=op.mult,
                    )
                    nc.sync.dma_start(
                        out=yv[n, :, :, c0 : c0 + HF], in_=q3
                    )
    nc.compile()
    _cache[key] = nc
    return nc


def _run(x, trace=False, **kwargs):
    from concourse import bass_utils

    nc = _build()
    xs = np.ascontiguousarray(x.reshape(N, C, HW))
    in_maps = [{"x": xs[i * NPC : (i + 1) * NPC]} for i in range(NCORES)]
    res = bass_utils.run_bass_kernel_spmd(
        nc, in_maps, core_ids=list(range(NCORES)), trace=trace, **kwargs
    )
    out = np.concatenate([r["y"] for r in res.results], axis=0)
    return np.asarray(out).astype(np.float32).reshape(N, C, H, W), res


def kernel(activations):
    out, _ = _run(np.asarray(activations))
    return out


# revision 6
# speedup vs baseline: 1.0618x; 1.0618x over previous
"""BFP activation quantization kernel for 8 TRN2 NeuronCores.

Reference semantics (per (n,h,w) block over C=256 channels):
    max_abs = max_c |x|;  e: max_abs = m*2^e, m in [0.5,1)  (frexp)
    delta = 2^(e-3);  q = trunc(x / delta) * delta

Sharding: pure data-parallel over N (batch): 64 images -> 8 per core.

Key insight: trunc(x/delta) keeps at most the top 2 mantissa bits of x
(|x/delta| < 8), so the bf16 truncation xb of x yields an EXACT result.
The whole datapath runs 16-bit after one 32-bit extraction pass:
  E1  bits' = (bits(x) >>a 16) | 1    (DVE, in-place i32). The |1 makes
      u = xb/delta never an exact integer (OR cannot carry, so sign,
      exponent and the block max's exponent are unchanged), which makes
      the single round-to-nearest convert below exact at trunc.
  E2  s16 = cvt_i16(bits')            (scalar engine; |v| < 2^15 exact)
  sb  = [s16 < 0]                     (DVE is_lt; int16 shifts of
      sign-extended values are NOT trustworthy on the 32-bit ALU lanes)
  a   = s16 & 0x7fff  -> fold halves with max -> ONE partition_all_reduce
      (max) per chunk at [128, HF] bf16. Folding first halves the Q7
      reduce width: the reduce cost is superlinear in width on HW and
      was the dominant bottleneck at [128, 2*HF].
  eb  = bits16(mx) & 0x7f80;  nr = -eb (= bits of -recip, exact int16
      negate since bits16(-2^k) = -bits16(2^k) mod 2^16);  db = eb-0x100
      (= bits of delta)
  t   = xb * (-recip)   (bf16 tensor_tensor, 2x DVE mode; the product is
      exact: 8-bit significand times a power of two)
  r   = cvt_i16(t + c), c = 0.5 - 2^-25  (scalar engine; equals ceil(t)
      because u is never an integer)
  w   = sb - r = trunc(u);  q = w * delta -> bf16 (exact: |w| <= 7
      times a power of two), converted to fp32 on the host (exact).
Verified bit-identical to the reference on the full input.
"""

import sys

for _p in ("/opt/trn_rl_repo", "/root/.axon_site/_ro/trn_rl_repo"):
    if _p not in sys.path:
        sys.path.append(_p)

import numpy as np

N, C, H, W = 64, 256, 56, 56
HW = H * W
NCORES = 8
NPC = N // NCORES  # images per core
F = 3136
HF = F // 2  # columns per chunk (2 chunks per image)
OB, OC, OD, OE, OF, OG, OH = 1, 2, 3, 4, 5, 6, 7  # stage offsets (chunks)
B_XT, B_S16, B_SB, B_RR, B_MXW, B_MXF, B_MFF, B_NR, B_DB = (
    4, 6, 5, 4, 3, 3, 3, 3, 5,
)

_cache = {}


def _build(repeat=1):
    key = ("v5", repeat)
    if key in _cache:
        return _cache[key]

    import concourse.bacc as bacc
    import concourse.mybir as mybir
    import concourse.tile as tile
    from concourse import bass_isa

    dt = mybir.dt
    op = mybir.AluOpType

    nc = bacc.Bacc(
        "TRN2",
        target_bir_lowering=False,
        debug=False,
        enable_asserts=False,
        num_devices=NCORES,
    )
    x_d = nc.dram_tensor("x", [NPC, C, HW], dt.float32, kind="ExternalInput").ap()
    y_d = nc.dram_tensor(
        "y", [NPC, C, HW], dt.bfloat16, kind="ExternalOutput"
    ).ap()

    NCH = NPC * 2 * repeat

    with tile.TileContext(nc) as tc:
        with (
            tc.tile_pool(name="xtp", bufs=B_XT) as xtp,
            tc.tile_pool(name="s16p", bufs=B_S16) as s16p,
            tc.tile_pool(name="sbp", bufs=B_SB) as sbp,
            tc.tile_pool(name="rrp", bufs=B_RR) as rrp,
            tc.tile_pool(name="mxwp", bufs=B_MXW) as mxwp,
            tc.tile_pool(name="mxfp", bufs=B_MXF) as mxfp,
            tc.tile_pool(name="mffp", bufs=B_MFF) as mffp,
            tc.tile_pool(name="nrp", bufs=B_NR) as nrp,
            tc.tile_pool(name="dbp", bufs=B_DB) as dbp,
            tc.tile_pool(name="consts", bufs=1) as consts,
        ):
            cbias = consts.tile([128, 1], dt.float32)  # +(0.5 - 2^-25)
            nc.gpsimd.memset(cbias[:], 0.4999999701976776123046875)

            xts, s16s, sbs, rrs, mxws, mxfs, mffs, nrs, dbs = (
                {}, {}, {}, {}, {}, {}, {}, {}, {},
            )

            def addr(k):
                n = (k // 2) % NPC
                c0 = (k % 2) * HF
                return n, c0

            yv = y_d.rearrange("n (j p) w -> n p j w", j=2)

            for k in range(NCH + OH):
                if OB <= k < NCH + OB:  # E1: (bits >>a 16) | 1 in place
                    j = k - OB
                    xi = xts[j][:].bitcast(dt.int32)
                    nc.vector.tensor_scalar(
                        out=xi, in0=xi, scalar1=16, scalar2=1,
                        op0=op.arith_shift_right, op1=op.bitwise_or,
                    )
                if OC <= k < NCH + OC:  # E2: narrow to i16 on scalar engine
                    j = k - OC
                    s16 = s16s[j] = s16p.tile(
                        [128, 2 * HF], dt.int16, tag="s16", name=f"s16_{j}"
                    )
                    nc.scalar.copy(out=s16[:], in_=xts[j][:].bitcast(dt.int32))
                if OD <= k < NCH + OD:  # sb, |bits|, fold, Q7 reduce
                    j = k - OD
                    s16 = s16s[j]
                    sb = sbs[j] = sbp.tile(
                        [128, 2 * HF], dt.int16, tag="sb", name=f"sb{j}"
                    )
                    nc.vector.tensor_scalar(
                        out=sb[:], in0=s16[:], scalar1=0, scalar2=None,
                        op0=op.is_lt,
                    )
                    mxw = mxws[j] = mxwp.tile(
                        [128, 2 * HF], dt.int16, tag="mxw", name=f"mxw{j}"
                    )
                    nc.vector.tensor_scalar(
                        out=mxw[:], in0=s16[:], scalar1=0x7FFF,
                        scalar2=None, op0=op.bitwise_and,
                    )
                    mff = mffs[j] = mffp.tile(
                        [128, HF], dt.int16, tag="mff", name=f"mff{j}"
                    )
                    nc.vector.tensor_tensor(
                        out=mff[:], in0=mxw[:, 0:HF],
                        in1=mxw[:, HF : 2 * HF], op=op.max,
                    )
                    mxf = mxfs[j] = mxfp.tile(
                        [128, HF], dt.bfloat16, tag="mxf", name=f"mxf{j}"
                    )
                    nc.gpsimd.partition_all_reduce(
                        mxf[:], mff[:].bitcast(dt.bfloat16), 128,
                        bass_isa.ReduceOp.max,
                    )
                if OE <= k < NCH + OE:  # eb, nr, db
                    j = k - OE
                    mxf = mxfs[j]
                    mffs.pop(j, None)
                    eb = mxf[:].bitcast(dt.int16)
                    nc.vector.tensor_scalar(
                        out=eb, in0=eb, scalar1=0x7F80, scalar2=None,
                        op0=op.bitwise_and,
                    )
                    nr = nrs[j] = nrp.tile(
                        [128, HF], dt.int16, tag="nr", name=f"nr{j}"
                    )
                    nc.scalar.mul(out=nr[:], in_=eb, mul=-1.0)
                    db = dbs[j] = dbp.tile(
                        [128, HF], dt.int16, tag="db", name=f"db{j}"
                    )
                    nc.vector.tensor_scalar(
                        out=db[:], in0=eb, scalar1=0x100, scalar2=None,
                        op0=op.subtract,
                    )
                if OF <= k < NCH + OF:  # t = xb * (-recip)
                    j = k - OF
                    s16, nr = s16s[j], nrs[j]
                    xb3 = s16[:].rearrange("p (j w) -> p j w", j=2).bitcast(
                        dt.bfloat16
                    )
                    nrf = (
                        nr[:].bitcast(dt.bfloat16)[:, None, :]
                        .broadcast_to([128, 2, HF])
                    )
                    nc.vector.tensor_tensor(
                        out=xb3, in0=xb3, in1=nrf, op=op.mult
                    )
                if OG <= k < NCH + OG:  # r = cvt_i16(t + c)
                    j = k - OG
                    rr = rrs[j] = rrp.tile(
                        [128, 2 * HF], dt.int16, tag="rr", name=f"rr{j}"
                    )
                    nc.scalar.activation(
                        out=rr[:], in_=s16s[j][:].bitcast(dt.bfloat16),
                        func=mybir.ActivationFunctionType.Identity,
                        bias=cbias[:], scale=1.0,
                    )
                if k < NCH:  # loads
                    n, c0 = addr(k)
                    xt = xts[k] = xtp.tile(
                        [128, 2 * HF], dt.float32, tag="xt", name=f"xt{k}"
                    )
                    nc.sync.dma_start(
                        out=xt[:, 0:HF], in_=x_d[n, 0:128, c0 : c0 + HF]
                    )
                    nc.sync.dma_start(
                        out=xt[:, HF : 2 * HF], in_=x_d[n, 128:256, c0 : c0 + HF]
                    )
                if OH <= k:  # w = sb - r ; q = w * delta ; store
                    j = k - OH
                    if j >= NCH:
                        continue
                    n, c0 = addr(j)
                    xts.pop(j, None)
                    s16s.pop(j)
                    sb = sbs.pop(j)
                    rr = rrs.pop(j)
                    mxws.pop(j, None)
                    mxfs.pop(j, None)
                    nrs.pop(j, None)
                    db = dbs.pop(j)
                    nc.vector.tensor_tensor(
                        out=sb[:], in0=sb[:], in1=rr[:], op=op.subtract
                    )
                    dbf = (
                        db[:].bitcast(dt.bfloat16)[:, None, :]
                        .broadcast_to([128, 2, HF])
                    )
                    q3 = rr[:].rearrange("p (j w) -> p j w", j=2).bitcast(
                        dt.bfloat16
                    )
                    nc.vector.tensor_tensor(
                        out=q3,
                        in0=sb[:].rearrange("p (j w) -> p j w", j=2),
                        in1=dbf, op=op.mult,
                    )
                    nc.sync.dma_start(
                        out=yv[n, :, :, c0 : c0 + HF], in_=q3
                    )
    nc.compile()
    _cache[key] = nc
    return nc


def _run(x, trace=False, **kwargs):
    from concourse import bass_utils

    nc = _build()
    xs = np.ascontiguousarray(x.reshape(N, C, HW))
    in_maps = [{"x": xs[i * NPC : (i + 1) * NPC]} for i in range(NCORES)]
    res = bass_utils.run_bass_kernel_spmd(
        nc, in_maps, core_ids=list(range(NCORES)), trace=trace, **kwargs
    )
    out = np.concatenate([r["y"] for r in res.results], axis=0)
    return np.asarray(out).astype(np.float32).reshape(N, C, H, W), res


def kernel(activations):
    out, _ = _run(np.asarray(activations))
    return out


# revision 9
# speedup vs baseline: 1.2323x; 1.1606x over previous
"""BFP activation quantization kernel for 8 TRN2 NeuronCores.

Reference semantics (per (n,h,w) block over C=256 channels):
    max_abs = max_c |x|;  e: max_abs = m*2^e, m in [0.5,1)  (frexp)
    delta = 2^(e-3);  q = trunc(x / delta) * delta

Sharding: pure data-parallel over N (batch): 64 images -> 8 per core.

Key insight: trunc(x/delta) keeps at most the top 2 mantissa bits of x
(|x/delta| < 8), so the bf16 truncation xb of x yields an EXACT result.
The whole datapath runs 16-bit after one 32-bit extraction pass:
  E1  bits' = (bits(x) >>a 16) | 1    (DVE, in-place i32). The |1 makes
      u = xb/delta never an exact integer (OR cannot carry, so sign,
      exponent and the block max's exponent are unchanged), which makes
      the single round-to-nearest convert below exact at trunc.
  E2  s16 = cvt_i16(bits')            (scalar engine; |v| < 2^15 exact)
  sb  = [s16 < 0]                     (DVE is_lt; int16 shifts of
      sign-extended values are NOT trustworthy on the 32-bit ALU lanes)
  a   = s16 & 0x7fff  -> fold halves with max -> ONE partition_all_reduce
      (max) per chunk at [128, HF] bf16. Folding first halves the Q7
      reduce width: the reduce cost is superlinear in width on HW and
      was the dominant bottleneck at [128, 2*HF].
  eb  = bits16(mx) & 0x7f80;  nr = -eb (= bits of -recip, exact int16
      negate since bits16(-2^k) = -bits16(2^k) mod 2^16);  db = eb-0x100
      (= bits of delta)
  t   = xb * (-recip)   (bf16 tensor_tensor, 2x DVE mode; the product is
      exact: 8-bit significand times a power of two)
  r   = cvt_i16(t + c), c = 0.5 - 2^-25  (scalar engine; equals ceil(t)
      because u is never an integer)
  w   = sb - r = trunc(u);  q = w * delta -> bf16 (exact: |w| <= 7
      times a power of two), converted to fp32 on the host (exact).
Verified bit-identical to the reference on the full input.
"""

import sys

for _p in ("/opt/trn_rl_repo", "/root/.axon_site/_ro/trn_rl_repo"):
    if _p not in sys.path:
        sys.path.append(_p)

import numpy as np

N, C, H, W = 64, 256, 56, 56
HW = H * W
NCORES = 8
NPC = N // NCORES  # images per core
F = 3136
HF = F // 2  # columns per chunk (2 chunks per image)
OB, OC, OD, OE, OF, OG, OH = 1, 2, 3, 4, 5, 6, 7  # stage offsets (chunks)
B_XT, B_S16, B_SB, B_RR, B_MXW, B_MXF, B_MFF, B_NR, B_DB = (
    4, 6, 5, 4, 3, 3, 3, 3, 5,
)

_cache = {}


def _build(repeat=1):
    key = ("v10", repeat)
    if key in _cache:
        return _cache[key]

    import concourse.bacc as bacc
    import concourse.mybir as mybir
    import concourse.tile as tile
    from concourse import bass_isa

    dt = mybir.dt
    op = mybir.AluOpType

    nc = bacc.Bacc(
        "TRN2",
        target_bir_lowering=False,
        debug=False,
        enable_asserts=False,
        num_devices=NCORES,
    )
    x_d = nc.dram_tensor("x", [NPC, C, HW], dt.float32, kind="ExternalInput").ap()
    y_d = nc.dram_tensor(
        "y", [NPC, C, HW], dt.bfloat16, kind="ExternalOutput"
    ).ap()

    NCH = NPC * 2 * repeat

    with tile.TileContext(nc) as tc:
        with (
            tc.tile_pool(name="xtp", bufs=B_XT) as xtp,
            tc.tile_pool(name="s16p", bufs=B_S16) as s16p,
            tc.tile_pool(name="sbp", bufs=B_SB) as sbp,
            tc.tile_pool(name="rrp", bufs=B_RR) as rrp,
            tc.tile_pool(name="mxwp", bufs=B_MXW) as mxwp,
            tc.tile_pool(name="mxfp", bufs=B_MXF) as mxfp,
            tc.tile_pool(name="mffp", bufs=B_MFF) as mffp,
            tc.tile_pool(name="nrp", bufs=B_NR) as nrp,
            tc.tile_pool(name="dbp", bufs=B_DB) as dbp,
            tc.tile_pool(name="consts", bufs=1) as consts,
        ):
            cbias = consts.tile([128, 1], dt.float32)  # +(0.5 - 2^-25)
            nc.gpsimd.memset(cbias[:], 0.4999999701976776123046875)

            xts, s16s, sbs, rrs, mxws, mxfs, mffs, nrs, dbs = (
                {}, {}, {}, {}, {}, {}, {}, {}, {},
            )

            def addr(k):
                n = (k // 2) % NPC
                c0 = (k % 2) * HF
                return n, c0

            yv = y_d.rearrange("n (j p) w -> n p j w", j=2)
            xv = x_d.rearrange("n (j p) w -> n p j w", j=2)

            for k in range(NCH + OH):
                if OB <= k < NCH + OB:  # E1: (bits >>a 16) | 1 in place
                    j = k - OB
                    xi = xts[j][:].bitcast(dt.int32)
                    nc.vector.tensor_scalar(
                        out=xi, in0=xi, scalar1=16, scalar2=1,
                        op0=op.arith_shift_right, op1=op.bitwise_or,
                    )
                if OC <= k < NCH + OC:  # E2: narrow to i16 on scalar engine
                    j = k - OC
                    s16 = s16s[j] = s16p.tile(
                        [128, 2 * HF], dt.int16, tag="s16", name=f"s16_{j}"
                    )
                    nc.scalar.copy(out=s16[:], in_=xts[j][:].bitcast(dt.int32))
                if OD <= k < NCH + OD:  # sb, |bits|, fold, Q7 reduce
                    j = k - OD
                    s16 = s16s[j]
                    sb = sbs[j] = sbp.tile(
                        [128, 2 * HF], dt.int16, tag="sb", name=f"sb{j}"
                    )
                    nc.vector.tensor_scalar(
                        out=sb[:], in0=s16[:], scalar1=0, scalar2=None,
                        op0=op.is_lt,
                    )
                    mxw = mxws[j] = mxwp.tile(
                        [128, 2 * HF], dt.int16, tag="mxw", name=f"mxw{j}"
                    )
                    nc.vector.tensor_scalar(
                        out=mxw[:], in0=s16[:], scalar1=0x7FFF,
                        scalar2=None, op0=op.bitwise_and,
                    )
                    mff = mffs[j] = mffp.tile(
                        [128, HF], dt.int16, tag="mff", name=f"mff{j}"
                    )
                    nc.vector.tensor_tensor(
                        out=mff[:], in0=mxw[:, 0:HF],
                        in1=mxw[:, HF : 2 * HF], op=op.max,
                    )
                    mxf = mxfs[j] = mxfp.tile(
                        [128, HF], dt.bfloat16, tag="mxf", name=f"mxf{j}"
                    )
                    nc.gpsimd.partition_all_reduce(
                        mxf[:], mff[:].bitcast(dt.bfloat16), 128,
                        bass_isa.ReduceOp.max,
                    )
                if OE <= k < NCH + OE:  # eb, nr, db
                    j = k - OE
                    mxf = mxfs[j]
                    mffs.pop(j, None)
                    eb = mxf[:].bitcast(dt.int16)
                    nc.vector.tensor_scalar(
                        out=eb, in0=eb, scalar1=0x7F80, scalar2=None,
                        op0=op.bitwise_and,
                    )
                    nr = nrs[j] = nrp.tile(
                        [128, HF], dt.int16, tag="nr", name=f"nr{j}"
                    )
                    nc.scalar.mul(out=nr[:], in_=eb, mul=-1.0)
                    db = dbs[j] = dbp.tile(
                        [128, HF], dt.int16, tag="db", name=f"db{j}"
                    )
                    nc.vector.tensor_scalar(
                        out=db[:], in0=eb, scalar1=0x100, scalar2=None,
                        op0=op.subtract,
                    )
                if OF <= k < NCH + OF:  # t = xb * (-recip)
                    j = k - OF
                    s16, nr = s16s[j], nrs[j]
                    xb3 = s16[:].rearrange("p (j w) -> p j w", j=2).bitcast(
                        dt.bfloat16
                    )
                    nrf = (
                        nr[:].bitcast(dt.bfloat16)[:, None, :]
                        .broadcast_to([128, 2, HF])
                    )
                    nc.vector.tensor_tensor(
                        out=xb3, in0=xb3, in1=nrf, op=op.mult
                    )
                if OG <= k < NCH + OG:  # r = cvt_i16(t + c)
                    j = k - OG
                    rr = rrs[j] = rrp.tile(
                        [128, 2 * HF], dt.int16, tag="rr", name=f"rr{j}"
                    )
                    nc.scalar.activation(
                        out=rr[:], in_=s16s[j][:].bitcast(dt.bfloat16),
                        func=mybir.ActivationFunctionType.Identity,
                        bias=cbias[:], scale=1.0,
                    )
                if k < NCH:  # loads
                    n, c0 = addr(k)
                    xt = xts[k] = xtp.tile(
                        [128, 2 * HF], dt.float32, tag="xt", name=f"xt{k}"
                    )
                    nc.sync.dma_start(
                        out=xt[:].rearrange("p (j w) -> p j w", j=2),
                        in_=xv[n, :, :, c0 : c0 + HF],
                    )
                if OH <= k:  # w = sb - r ; q = w * delta ; store
                    j = k - OH
                    if j >= NCH:
                        continue
                    n, c0 = addr(j)
                    xts.pop(j, None)
                    s16s.pop(j)
                    sb = sbs.pop(j)
                    rr = rrs.pop(j)
                    mxws.pop(j, None)
                    mxfs.pop(j, None)
                    nrs.pop(j, None)
                    db = dbs.pop(j)
                    nc.vector.tensor_tensor(
                        out=sb[:], in0=sb[:], in1=rr[:], op=op.subtract
                    )
                    dbf = (
                        db[:].bitcast(dt.bfloat16)[:, None, :]
                        .broadcast_to([128, 2, HF])
                    )
                    q3 = rr[:].rearrange("p (j w) -> p j w", j=2).bitcast(
                        dt.bfloat16
                    )
                    nc.vector.tensor_tensor(
                        out=q3,
                        in0=sb[:].rearrange("p (j w) -> p j w", j=2),
                        in1=dbf, op=op.mult,
                    )
                    nc.sync.dma_start(
                        out=yv[n, :, :, c0 : c0 + HF], in_=q3
                    )
    nc.compile()
    _cache[key] = nc
    return nc


def _run(x, trace=False, **kwargs):
    from concourse import bass_utils

    nc = _build()
    xs = np.ascontiguousarray(x.reshape(N, C, HW))
    in_maps = [{"x": xs[i * NPC : (i + 1) * NPC]} for i in range(NCORES)]
    res = bass_utils.run_bass_kernel_spmd(
        nc, in_maps, core_ids=list(range(NCORES)), trace=trace, **kwargs
    )
    out = np.concatenate([r["y"] for r in res.results], axis=0)
    return np.asarray(out).astype(np.float32).reshape(N, C, H, W), res


def kernel(activations):
    out, _ = _run(np.asarray(activations))
    return out
